# revision 2
# baseline (speedup 1.0000x reference)
"""Trainium2 Bass kernel: batched int8 GEMM (bmm_s8t_s8n) with fused bf16 dequant.

Computes out[i] = bf16(alpha * (a[i] @ b[i]^T)) for a [32,512,2048] int8,
b [32,512,2048] int8 (both row-major with K innermost), alpha scalar fp32.

Strategy (per 8-core SPMD shard = 4 batches/core):
  1. The TensorE contracts over the partition axis, so both operands need K on
     partitions.  DMA xbar transpose only supports 2-byte elements, so we view
     int8 pairs along K as uint16 and transpose [512, 128]-uint16 column chunks
     of each batch matrix into SBUF tiles [128, 512]-uint16.  Partition j of
     chunk c then holds the two int8 k-slices k=256c+2j and k=256c+2j+1
     byte-interleaved along the free dim.
  2. int8 -> bf16 conversion copies (DVE for `a`, ACT for `b`) de-interleave
     the two parities into a [128, 1024] bf16 tile whose halves are the two
     k-slices.  int8 values are exact in bf16; products accumulate exactly in
     fp32 PSUM (|acc| << 2^24), so results match int32 accumulation bit-wise.
  3. 16 accumulating matmuls per output tile: psum[128m, 512n] += aT.T @ bT.
  4. Dequant: ACT/DVE copy PSUM->SBUF with scale=alpha, cast to bf16, DMA out.
"""

import numpy as np

import concourse.mybir as mybir
from concourse import bacc
from concourse.bass_utils import run_bass_kernel_spmd
from concourse.tile import TileContext

B, M, N, K = 32, 512, 512, 2048
NCORES = 8
BPC = B // NCORES  # batches per core
KP = K // 2  # uint16 pair-columns per row
PART = 128
NCHUNK = KP // PART  # transposed chunks per operand-batch (8)
KTILES = 2 * NCHUNK  # k-tiles of 128 per batch (16)

STAGE_BUFS = 12  # per tag: uint16 staging tiles [128, 512] u16 (1 KiB/partition)
CONV_BUFS = 3 * NCHUNK  # per tag: bf16 k-tiles [128, 1024] bf16 (2 KiB/partition)


def _build(alpha: float, bpc: int = BPC):
    nc = bacc.Bacc("TRN2", target_bir_lowering=False)
    a_d = nc.dram_tensor("a", [bpc, M, KP], mybir.dt.uint16, kind="ExternalInput")
    b_d = nc.dram_tensor("b", [bpc, N, KP], mybir.dt.uint16, kind="ExternalInput")
    o_d = nc.dram_tensor("out", [bpc, M, N], mybir.dt.bfloat16, kind="ExternalOutput")

    with TileContext(nc) as tc:
        with (
            tc.tile_pool(name="stage", bufs=STAGE_BUFS) as stage,
            tc.tile_pool(name="conv", bufs=CONV_BUFS) as conv,
            tc.tile_pool(name="obuf", bufs=8) as obuf,
            tc.tile_pool(name="psum", bufs=8, space="PSUM") as psum_pool,
        ):
            for bi in range(bpc):
                ktiles = {"a": [], "b": []}
                for c in range(NCHUNK):
                    for name, dram in (("a", a_d), ("b", b_d)):
                        st = stage.tile([PART, M], mybir.dt.uint16, tag=f"st_{name}")
                        nc.sync.dma_start_transpose(
                            st[:, :], dram[bi, :, c * PART : (c + 1) * PART]
                        )
                        # [128, 1024] int8 view: free byte 2m+p = x[m, 256c+2j+p]
                        st8 = st[:, :].bitcast(mybir.dt.int8)
                        bt = conv.tile([PART, 2 * M], mybir.dt.bfloat16, tag=f"bf_{name}")
                        eng = nc.vector if name == "a" else nc.scalar
                        if name == "a":
                            eng.tensor_copy(
                                out=bt[:, :].rearrange("q (p m) -> q p m", m=M),
                                in_=st8.rearrange("q (m p) -> q p m", p=2),
                            )
                        else:
                            eng.copy(
                                out=bt[:, :].rearrange("q (p m) -> q p m", m=M),
                                in_=st8.rearrange("q (m p) -> q p m", p=2),
                            )
                        ktiles[name].append(bt)
                for mi in range(M // PART):
                    ps = psum_pool.tile([PART, N], mybir.dt.float32)
                    t = 0
                    for c in range(NCHUNK):
                        for p in range(2):
                            nc.tensor.matmul(
                                ps[:, :],
                                ktiles["a"][c][:, p * M + mi * PART : p * M + (mi + 1) * PART],
                                ktiles["b"][c][:, p * N : (p + 1) * N],
                                start=(t == 0),
                                stop=(t == KTILES - 1),
                            )
                            t += 1
                    ot = obuf.tile([PART, N], mybir.dt.bfloat16)
                    nc.scalar.activation(
                        ot[:, :],
                        ps[:, :],
                        mybir.ActivationFunctionType.Copy,
                        scale=float(alpha),
                    )
                    nc.scalar.dma_start(o_d[bi, mi * PART : (mi + 1) * PART, :], ot[:, :])
    nc.compile()
    return nc


def run(a, b, alpha, trace=False):
    """Run on 8 NeuronCores; returns (out [32,512,512] bf16, BassKernelResults)."""
    a = np.ascontiguousarray(a)
    b = np.ascontiguousarray(b)
    assert a.dtype == np.int8 and b.dtype == np.int8, (a.dtype, b.dtype)
    nc = _build(float(alpha))
    in_maps = []
    for ci in range(NCORES):
        sl = slice(ci * BPC, (ci + 1) * BPC)
        in_maps.append({"a": a[sl].view(np.uint16), "b": b[sl].view(np.uint16)})
    res = run_bass_kernel_spmd(nc, in_maps, core_ids=list(range(NCORES)), trace=trace)
    out = np.concatenate([r["out"] for r in res.results], axis=0)
    return out, res


def kernel(a, b, alpha):
    out, _ = run(a, b, alpha)
    return out


# revision 5
# speedup vs baseline: 1.7790x; 1.7790x over previous
"""Trainium2 Bass kernel: batched int8 GEMM (bmm_s8t_s8n) with fused bf16 dequant.

Computes out[i] = bf16(alpha * (a[i] @ b[i]^T)) for a [32,512,2048] int8,
b [32,512,2048] int8 (both row-major with K innermost), alpha scalar fp32.

Strategy (per 8-core SPMD shard = 4 batches/core):
  1. The TensorE contracts over the partition axis, so both operands need K on
     partitions.  DMA xbar transpose only supports 2-byte elements, so we view
     int8 pairs along K as uint16 and transpose [512, 128]-uint16 column chunks
     of each batch matrix into SBUF tiles [128, 512]-uint16.  Partition j of
     chunk c then holds the two int8 k-slices k=256c+2j and k=256c+2j+1
     byte-interleaved along the free dim.
  2. int8 -> bf16 conversion copies (DVE for `a`, ACT for `b`) de-interleave
     the two parities into a [128, 1024] bf16 tile whose halves are the two
     k-slices.  int8 values are exact in bf16; products accumulate exactly in
     fp32 PSUM (|acc| << 2^24), so results match int32 accumulation bit-wise.
  3. 16 accumulating matmuls per output tile: psum[128m, 512n] += aT.T @ bT.
  4. Dequant: ACT/DVE copy PSUM->SBUF with scale=alpha, cast to bf16, DMA out.
"""

import numpy as np

import concourse.mybir as mybir
from concourse import bacc
from concourse.bass_utils import run_bass_kernel_spmd
from concourse.tile import TileContext

B, M, N, K = 32, 512, 512, 2048
NCORES = 8
BPC = B // NCORES  # batches per core
KP = K // 2  # uint16 pair-columns per row
PART = 128
NCHUNK = KP // PART  # transposed chunks per operand-batch (8)
KTILES = 2 * NCHUNK  # k-tiles of 128 per batch (16)

STAGE_BUFS = 3  # per tag: uint16 staging tiles [128, KP] u16 (8 KiB/partition)
CONV_BUFS = 20  # per tag: bf16 k-tiles [128, 1024] bf16 (2 KiB/partition)


def _build(alpha: float, bpc: int = BPC):
    nc = bacc.Bacc("TRN2", target_bir_lowering=False)
    a_d = nc.dram_tensor("a", [bpc, M, KP], mybir.dt.uint16, kind="ExternalInput")
    b_d = nc.dram_tensor("b", [bpc, N, KP], mybir.dt.uint16, kind="ExternalInput")
    o_d = nc.dram_tensor("out", [bpc, M, N], mybir.dt.bfloat16, kind="ExternalOutput")

    with TileContext(nc) as tc:
        with (
            tc.tile_pool(name="stage", bufs=STAGE_BUFS) as stage,
            tc.tile_pool(name="conv", bufs=CONV_BUFS) as conv,
            tc.tile_pool(name="obuf", bufs=8) as obuf,
            tc.tile_pool(name="psum", bufs=8, space="PSUM") as psum_pool,
        ):
            for bi in range(bpc):
                ktiles = {"a": [], "b": []}
                sts = {}
                for name, dram in (("a", a_d), ("b", b_d)):
                    # One whole-matrix xbar transpose [512, 1024]u16 -> [128, 8, 512].
                    # Whatever (partition, chunk) <-> column mapping the xbar uses,
                    # it is identical for a and b, so the contraction enumerates
                    # every k exactly once — correctness does not depend on it.
                    st = stage.tile([PART, NCHUNK * M], mybir.dt.uint16, tag=f"st_{name}")
                    nc.sync.dma_start_transpose(
                        st[:, :].rearrange("q (c m) -> q c m", m=M), dram[bi]
                    )
                    sts[name] = st[:, :].bitcast(mybir.dt.int8)  # [128, 2*KP]
                for c in range(NCHUNK):
                    for name in ("a", "b"):
                        chunk8 = sts[name][:, c * 2 * M : (c + 1) * 2 * M]
                        bt = conv.tile([PART, 2 * M], mybir.dt.bfloat16, tag=f"bf_{name}")
                        if name == "a":
                            nc.vector.tensor_copy(
                                out=bt[:, :].rearrange("q (p m) -> q p m", m=M),
                                in_=chunk8.rearrange("q (m p) -> q p m", p=2),
                            )
                        else:
                            nc.scalar.copy(
                                out=bt[:, :].rearrange("q (p m) -> q p m", m=M),
                                in_=chunk8.rearrange("q (m p) -> q p m", p=2),
                            )
                        ktiles[name].append(bt)
                for mi in range(M // PART):
                    ps = psum_pool.tile([PART, N], mybir.dt.float32)
                    t = 0
                    for c in range(NCHUNK):
                        for p in range(2):
                            nc.tensor.matmul(
                                ps[:, :],
                                ktiles["a"][c][:, p * M + mi * PART : p * M + (mi + 1) * PART],
                                ktiles["b"][c][:, p * N : (p + 1) * N],
                                start=(t == 0),
                                stop=(t == KTILES - 1),
                            )
                            t += 1
                    ot = obuf.tile([PART, N], mybir.dt.bfloat16)
                    nc.vector.tensor_scalar_mul(ot[:, :], ps[:, :], float(alpha))
                    nc.gpsimd.dma_start(o_d[bi, mi * PART : (mi + 1) * PART, :], ot[:, :])
    nc.compile()
    return nc


def run(a, b, alpha, trace=False):
    """Run on 8 NeuronCores; returns (out [32,512,512] bf16, BassKernelResults)."""
    a = np.ascontiguousarray(a)
    b = np.ascontiguousarray(b)
    assert a.dtype == np.int8 and b.dtype == np.int8, (a.dtype, b.dtype)
    nc = _build(float(alpha))
    in_maps = []
    for ci in range(NCORES):
        sl = slice(ci * BPC, (ci + 1) * BPC)
        in_maps.append({"a": a[sl].view(np.uint16), "b": b[sl].view(np.uint16)})
    res = run_bass_kernel_spmd(nc, in_maps, core_ids=list(range(NCORES)), trace=trace)
    out = np.concatenate([r["out"] for r in res.results], axis=0)
    return out, res


def kernel(a, b, alpha):
    out, _ = run(a, b, alpha)
    return out


# revision 8
# speedup vs baseline: 1.8082x; 1.0164x over previous
"""Trainium2 Bass kernel: batched int8 GEMM (bmm_s8t_s8n) with fused bf16 dequant.

Computes out[i] = bf16(alpha * (a[i] @ b[i]^T)) for a [32,512,2048] int8,
b [32,512,2048] int8 (both row-major with K innermost), alpha scalar fp32.

Strategy (per 8-core SPMD shard = 4 batches/core):
  1. The TensorE contracts over the partition axis, so both operands need K on
     partitions.  DMA xbar transpose only supports 2-byte elements, so we view
     int8 pairs along K as uint16 and transpose [512, 128]-uint16 column chunks
     of each batch matrix into SBUF tiles [128, 512]-uint16.  Partition j of
     chunk c then holds the two int8 k-slices k=256c+2j and k=256c+2j+1
     byte-interleaved along the free dim.
  2. int8 -> bf16 conversion copies (DVE for `a`, ACT for `b`) de-interleave
     the two parities into a [128, 1024] bf16 tile whose halves are the two
     k-slices.  int8 values are exact in bf16; products accumulate exactly in
     fp32 PSUM (|acc| << 2^24), so results match int32 accumulation bit-wise.
  3. 16 accumulating matmuls per output tile: psum[128m, 512n] += aT.T @ bT.
  4. Dequant: ACT/DVE copy PSUM->SBUF with scale=alpha, cast to bf16, DMA out.
"""

import numpy as np

import concourse.mybir as mybir
from concourse import bacc
from concourse.bass_utils import run_bass_kernel_spmd
from concourse.tile import TileContext

B, M, N, K = 32, 512, 512, 2048
NCORES = 8
BPC = B // NCORES  # batches per core
KP = K // 2  # uint16 pair-columns per row
PART = 128
NCHUNK = KP // PART  # transposed chunks per operand-batch (8)
KTILES = 2 * NCHUNK  # k-tiles of 128 per batch (16)

STAGE_BUFS = 3  # per tag: uint16 staging tiles [128, KP] u16 (8 KiB/partition)
CONV_BUFS = 20  # per tag: bf16 k-tiles [128, 1024] bf16 (2 KiB/partition)


def _build(alpha: float, bpc: int = BPC):
    nc = bacc.Bacc("TRN2", target_bir_lowering=False)
    a_d = nc.dram_tensor("a", [bpc, M, KP], mybir.dt.uint16, kind="ExternalInput")
    b_d = nc.dram_tensor("b", [bpc, N, KP], mybir.dt.uint16, kind="ExternalInput")
    o_d = nc.dram_tensor("out", [bpc, M, N], mybir.dt.bfloat16, kind="ExternalOutput")

    with TileContext(nc) as tc:
        with (
            tc.tile_pool(name="stage", bufs=STAGE_BUFS) as stage,
            tc.tile_pool(name="conv", bufs=CONV_BUFS) as conv,
            tc.tile_pool(name="obuf", bufs=8) as obuf,
            tc.tile_pool(name="psum", bufs=8, space="PSUM") as psum_pool,
        ):
            for bi in range(bpc):
                ktiles = {"a": [], "b": []}
                sts = {}
                for name, dram, ring in (("a", a_d, nc.sync), ("b", b_d, nc.sync)):
                    # One whole-matrix xbar transpose [512, 1024]u16 -> [128, 8, 512].
                    # Whatever (partition, chunk) <-> column mapping the xbar uses,
                    # it is identical for a and b, so the contraction enumerates
                    # every k exactly once — correctness does not depend on it.
                    st = stage.tile([PART, NCHUNK * M], mybir.dt.uint16, tag=f"st_{name}")
                    ring.dma_start_transpose(
                        st[:, :].rearrange("q (c m) -> q c m", m=M), dram[bi]
                    )
                    sts[name] = st[:, :].bitcast(mybir.dt.int8)  # [128, 2*KP]
                for c in range(NCHUNK):
                    for name in ("a", "b"):
                        chunk8 = sts[name][:, c * 2 * M : (c + 1) * 2 * M]
                        bt = conv.tile([PART, 2 * M], mybir.dt.bfloat16, tag=f"bf_{name}")
                        if name == "a":
                            eng = nc.vector
                        else:
                            eng = nc.scalar
                        if eng is nc.scalar:
                            eng.copy(
                                out=bt[:, :].rearrange("q (p m) -> q p m", m=M),
                                in_=chunk8.rearrange("q (m p) -> q p m", p=2),
                            )
                        else:
                            eng.tensor_copy(
                                out=bt[:, :].rearrange("q (p m) -> q p m", m=M),
                                in_=chunk8.rearrange("q (m p) -> q p m", p=2),
                            )
                        ktiles[name].append(bt)
                for mi in range(M // PART):
                    ps = psum_pool.tile([PART, N], mybir.dt.float32)
                    t = 0
                    for c in range(NCHUNK):
                        for p in range(2):
                            nc.tensor.matmul(
                                ps[:, :],
                                ktiles["a"][c][:, p * M + mi * PART : p * M + (mi + 1) * PART],
                                ktiles["b"][c][:, p * N : (p + 1) * N],
                                start=(t == 0),
                                stop=(t == KTILES - 1),
                            )
                            t += 1
                    ot = obuf.tile([PART, N], mybir.dt.bfloat16)
                    nc.vector.tensor_scalar_mul(ot[:, :], ps[:, :], float(alpha))
                    nc.gpsimd.dma_start(o_d[bi, mi * PART : (mi + 1) * PART, :], ot[:, :])
    nc.compile()
    return nc


def run(a, b, alpha, trace=False):
    """Run on 8 NeuronCores; returns (out [32,512,512] bf16, BassKernelResults)."""
    a = np.ascontiguousarray(a)
    b = np.ascontiguousarray(b)
    assert a.dtype == np.int8 and b.dtype == np.int8, (a.dtype, b.dtype)
    nc = _build(float(alpha))
    in_maps = []
    for ci in range(NCORES):
        sl = slice(ci * BPC, (ci + 1) * BPC)
        in_maps.append({"a": a[sl].view(np.uint16), "b": b[sl].view(np.uint16)})
    res = run_bass_kernel_spmd(nc, in_maps, core_ids=list(range(NCORES)), trace=trace)
    out = np.concatenate([r["out"] for r in res.results], axis=0)
    return out, res


def kernel(a, b, alpha):
    out, _ = run(a, b, alpha)
    return out
